# revision 26
# baseline (speedup 1.0000x reference)
"""Distributed masked-attention kernel for 8 TRN2 NeuronCores (v5).

Reference computation (B=2, L=1024, D=1024, H=16, DH=64):
    Qz, Kz = masked Q, K;  Qp/Kp/Vp = projections (V = K)
    per-head attention with outer-product validity mask, softmax scaled
    by 1/sqrt(D);  O = Qp + attn;  out = O + relu(mask_q(O @ Wo.T))

Sharding: core c = 2*g + b handles batch b = c%2, head group g = c//2
(4 heads, feature block e = [256g, 256g+256)).  Activations are
feature-major ("X.T" = [features, tokens]); host pre-transposes,
pre-zeroes masked rows, and casts to bf16.

v5: no collectives.  Each core emits its O block (for the residual)
and its partial output projection O_c @ Wo[:, block].T; the host sums
the four feature-block partials per batch and applies relu + residual
while unsharding.  This removes the cross-core barrier (~50us of ncfw
startup) and the serial AllToAll chain from the device critical path.

  - projections accumulate in 8 PSUM banks and consume input-DMA
    chunks as they arrive (dc-pipelined).
  - ACT does only exp + denominator-row copies; reciprocals on DVE.
  - partial output projection per q-chunk (16 MMs) overlaps the next
    attention chunk; outputs stream out as they are produced.
"""
import numpy as np

B, L, D = 2, 1024, 1024
H, DH = 16, 64
NCORES = 8
HPC = 4          # heads per core
EB = 256         # feature block per core
NEG = -30000.0   # masked-k bias (exp -> exact 0)
BIG = 1e30       # masked-q denominator prefill

TRACE = False
TRACE_KWARGS = {}
LAST_RESULTS = None

_compiled = None


def _build():
    import concourse.bacc as bacc
    import concourse.tile as tile
    from concourse import mybir

    f32 = mybir.dt.float32
    bf16 = mybir.dt.bfloat16
    f8 = mybir.dt.float8e4
    DR = mybir.MatmulPerfMode.DoubleRow
    ALU = mybir.AluOpType
    EXP = mybir.ActivationFunctionType.Exp
    CPY = mybir.ActivationFunctionType.Copy

    nc = bacc.Bacc("TRN2", target_bir_lowering=False, debug=False,
                   num_devices=NCORES)

    # inputs pre-relaid on host to [128, dc*F] so each loads as one
    # contiguous full-partition DMA
    qt = nc.dram_tensor("qt", [128, 8 * L], bf16, kind="ExternalInput")
    kt = nc.dram_tensor("kt", [128, 8 * L], f8, kind="ExternalInput")
    wq = nc.dram_tensor("wq", [128, 8 * EB], bf16, kind="ExternalInput")
    wk = nc.dram_tensor("wk", [128, 8 * EB], f8, kind="ExternalInput")
    wv = nc.dram_tensor("wv", [128, 8 * EB], f8, kind="ExternalInput")
    wos = nc.dram_tensor("wos", [EB, D], bf16, kind="ExternalInput")
    bk = nc.dram_tensor("bk", [128, 8], f32, kind="ExternalInput")
    mvn = nc.dram_tensor("mvn", [1, L], bf16, kind="ExternalInput")
    pout = nc.dram_tensor("pout", [D, L], bf16, kind="ExternalOutput")
    oout = nc.dram_tensor("oout", [EB, L], bf16, kind="ExternalOutput")

    with tile.TileContext(nc) as tc:
        with (
            tc.tile_pool(name="sb", bufs=1) as sb,
            tc.tile_pool(name="rot", bufs=4) as rot,
            tc.tile_pool(name="ps_big", bufs=2, space="PSUM") as ps_big,
            tc.tile_pool(name="ps_at", bufs=4, space="PSUM") as ps_at,
        ):
            # ---- constants / masks (tiny, on the otherwise-idle SWDGE) ----
            bk_t = sb.tile([128, 8], f32, tag="bk")
            mvn_t = sb.tile([1, L], bf16, tag="mvn")
            nc.gpsimd.dma_start(bk_t[:], bk[:])
            nc.gpsimd.dma_start(mvn_t[:], mvn[:])
            # masked-q denominator prefill row: BIG at masked q, 0 elsewhere
            bm_t = sb.tile([1, L], f32, tag="bm")
            nc.vector.tensor_scalar_mul(bm_t[:], mvn_t[:], BIG)

            # ---- warmup: PE matmuls + ACT exp-table load, no input deps ----
            warm_w = sb.tile([128, 128], bf16, tag="warmw")
            warm_x = sb.tile([128, 512], bf16, tag="warmx")
            nc.vector.memset(warm_w[:], 0.5)
            nc.vector.memset(warm_x[:], 0.5)
            warm_act = rot.tile([128, 512], bf16, tag="p", name="warm_act")
            nc.scalar.activation(warm_act[:], warm_x[:], EXP, scale=0.25)
            warm_ps = ps_big.tile([128, 1024], f32, tag="big", name="warm_ps")
            for w in range(9):
                nc.tensor.matmul(warm_ps[:, 0:512], warm_w[:], warm_x[:],
                                 start=(w == 0), stop=(w == 8))

            # ---- input DMAs: one contiguous transfer per tensor,
            # qt split in dc-halves for pipelining ----
            qt_all = sb.tile([128, 8 * L], bf16, tag="qta")
            kt_all = sb.tile([128, 8 * L], f8, tag="kta")
            wq_all = sb.tile([128, 8 * EB], bf16, tag="wqa")
            wk_all = sb.tile([128, 8 * EB], f8, tag="wka")
            wv_all = sb.tile([128, 8 * EB], f8, tag="wva")
            nc.gpsimd.dma_start(wk_all[:], wk[:])
            nc.gpsimd.dma_start(wv_all[:], wv[:])
            nc.sync.dma_start(wq_all[:], wq[:])
            nc.sync.dma_start(qt_all[:, 0:4 * L], qt[:, 0:4 * L])
            nc.scalar.dma_start(kt_all[:], kt[:])
            nc.scalar.dma_start(qt_all[:, 4 * L:8 * L], qt[:, 4 * L:8 * L])
            # epilogue weight shard [256, 1024] (sync FIFO, after inputs)
            wos_t = [sb.tile([128, D], bf16, tag=f"wos{i}", name=f"wos{i}")
                     for i in range(2)]
            for i in range(2):
                nc.sync.dma_start(wos_t[i][:], wos[128 * i:128 * (i + 1), :])

            qt_t = [qt_all[:, L * i:L * (i + 1)] for i in range(8)]
            kt_t = [kt_all[:, L * i:L * (i + 1)] for i in range(8)]
            wq_t = [wq_all[:, EB * i:EB * (i + 1)] for i in range(8)]
            wk_t = [wk_all[:, EB * i:EB * (i + 1)] for i in range(8)]
            wv_t = [wv_all[:, EB * i:EB * (i + 1)] for i in range(8)]

            # ---- phase 1: projections, dc-pipelined, PSUM-resident ----
            qp0 = ps_big.tile([128, 1024], f32, tag="big", name="qp0")
            kp0 = ps_big.tile([128, 1024], f32, tag="big", name="kp0")
            qp1 = [ps_at.tile([128, 512], f32, tag="at", name=f"qp1_{qc}")
                   for qc in range(2)]
            kp1 = [ps_at.tile([128, 512], f32, tag="at", name=f"kp1_{qc}")
                   for qc in range(2)]
            # Qp bf16, Kp fp8 (normal mode — fp8 runs at bf16 speed, which
            # keeps the PE dense enough for HAM to stay un-throttled; the
            # fp8 win here is input-DMA bytes, not matmul slots).
            for dc in range(8):
                st = (dc == 0)
                sp = (dc == 7)
                for qc in range(2):
                    qs = slice(512 * qc, 512 * (qc + 1))
                    nc.tensor.matmul(kp0[:, qs], wk_t[dc][:, 0:128],
                                     kt_t[dc][:, qs], start=st, stop=sp)
                for qc in range(2):
                    qs = slice(512 * qc, 512 * (qc + 1))
                    nc.tensor.matmul(kp1[qc][:], wk_t[dc][:, 128:256],
                                     kt_t[dc][:, qs], start=st, stop=sp)
                for qc in range(2):
                    qs = slice(512 * qc, 512 * (qc + 1))
                    nc.tensor.matmul(qp0[:, qs], wq_t[dc][:, 0:128],
                                     qt_t[dc][:, qs], start=st, stop=sp)
                for qc in range(2):
                    qs = slice(512 * qc, 512 * (qc + 1))
                    nc.tensor.matmul(qp1[qc][:], wq_t[dc][:, 128:256],
                                     qt_t[dc][:, qs], start=st, stop=sp)

            # PSUM -> SBUF (inputs pre-zeroed on host, plain copies)
            qpt = [sb.tile([128, L], bf16, tag=f"qpt{i}", name=f"qpt{i}")
                   for i in range(2)]
            kpt = [sb.tile([128, L], bf16, tag=f"kpt{i}", name=f"kpt{i}")
                   for i in range(2)]
            nc.vector.tensor_copy(kpt[0][:, :], kp0[:])
            nc.vector.tensor_copy(qpt[0][:, :], qp0[:])
            for qc in range(2):
                qs = slice(512 * qc, 512 * (qc + 1))
                nc.vector.tensor_copy(kpt[1][:, qs], kp1[qc][:])
                nc.vector.tensor_copy(qpt[1][:, qs], qp1[qc][:])

            # Vp in fp8 DoubleRow pair layout: vpa8[t][p, h, j, m] holds V
            # for k-token pair-chunks (2t+j); m stride padded to 80 B so the
            # DoubleRow weight AP satisfies step%16==0.  Column m=64 is the
            # softmax-denominator ones column (memset covers it).
            # V is stored x32 in fp8 (the raw 1/sqrt(D)-scaled values sit
            # in e4m3's subnormal range); the ones column is 32.0 so the
            # softmax normalization cancels the scale exactly.
            vpa8 = [sb.tile([128, HPC * 2 * 80], f8, tag=f"vpa{i}",
                            name=f"vpa{i}") for i in range(4)]
            for t in range(4):
                nc.gpsimd.memset(vpa8[t][:], 32.0)

            def vproj(tt):
                pv = ps_at.tile([128, EB], f32, tag="at", name=f"pv{tt}")
                for dc in range(8):
                    nc.tensor.matmul(
                        pv[:], kt_t[dc][:, 128 * tt:128 * (tt + 1)],
                        wv_t[dc][:], start=(dc == 0), stop=(dc == 7))
                # one strided fp8 cast per chunk: 4 heads at 80-stride
                dstv = vpa8[tt // 2][:].rearrange(
                    "p (h j m) -> p h j m", h=HPC, j=2)[:, :, tt % 2, 0:64]
                nc.vector.tensor_copy(dstv, pv[:].rearrange(
                    "p (h m) -> p h m", h=HPC))

            vproj(0)
            vproj(1)

            attn = [sb.tile([128, L], bf16, tag=f"attn{i}", name=f"attn{i}")
                    for i in range(2)]
            ot = [sb.tile([128, L], bf16, tag=f"ot{i}", name=f"ot{i}")
                  for i in range(2)]
            po = [sb.tile([128, 4096], bf16, tag=f"po{i}", name=f"po{i}")
                  for i in range(2)]

            # one partial-output-projection step (2 MMs + copy + DMA);
            # interleaved into later attention slots as PE gap filler
            def epi(qc, ec, on_act):
                qs = slice(512 * qc, 512 * (qc + 1))
                fpt = ps_at.tile([128, 512], f32, tag="at",
                                 name=f"fp{qc}_{ec}")
                for dc in range(2):
                    nc.tensor.matmul(
                        fpt[:], wos_t[dc][:, 128 * ec:128 * (ec + 1)],
                        ot[dc][:, qs], start=(dc == 0), stop=(dc == 1))
                pslice = po[qc][:, 512 * ec:512 * (ec + 1)]
                if on_act:
                    nc.scalar.activation(pslice, fpt[:], CPY)
                else:
                    nc.vector.tensor_copy(pslice, fpt[:])
                nc.sync.dma_start(pout[128 * ec:128 * (ec + 1), qs], pslice)

            # ---- phase 2+3: attention q-chunk-major; local partial
            # output projection per q-chunk ----
            for qc in range(2):
                qs = slice(512 * qc, 512 * (qc + 1))
                for hp in (0, 2):
                    et = hp // 2
                    ats = [ps_at.tile([65, 512], f32, tag="at",
                                      name=f"at{h}_{qc}")
                           for h in (hp, hp + 1)]
                    # software pipeline; exp writes fp8 pair tiles, AV runs
                    # fp8 DoubleRow (contracts 256 k per MM, 2 MMs per pair)
                    def av(t, p2, stop):
                        for jh in range(2):
                            lhsT = vpa8[t][:].rearrange(
                                "p (h j m) -> p h j m", h=HPC, j=2)[
                                :, hp + jh, :, 0:65]
                            rhs = p2[:].rearrange(
                                "p (h j q) -> p h j q", h=2, j=2)[:, jh]
                            nc.tensor.matmul(ats[jh][:], lhsT, rhs,
                                             start=(t == 0), stop=stop,
                                             perf_mode=DR)

                    p2_tiles = {}
                    for ki in range(8):
                        t, jj = ki // 2, ki % 2
                        ks = slice(128 * ki, 128 * (ki + 1))
                        s_ps = ps_big.tile([128, 1024], f32, tag="big",
                                           name=f"s{hp}_{qc}_{ki}")
                        for j in range(2):
                            ro = 64 * j
                            nc.tensor.matmul(
                                s_ps[:, 512 * j:512 * (j + 1)],
                                kpt[et][ro:ro + 64, ks],
                                qpt[et][ro:ro + 64, qs],
                                start=True, stop=True)
                        if jj == 0:
                            p2_tiles[t] = rot.tile([128, 2048], f8, tag="p",
                                                   name=f"p{hp}_{qc}_{t}")
                        dst = p2_tiles[t][:].rearrange(
                            "p (h j q) -> p h j q", h=2, j=2)[:, :, jj, :]
                        # kpt carries a x32 scale (wk stored x32 in fp8)
                        nc.scalar.activation(dst, s_ps[:], EXP,
                                             bias=bk_t[:, ki:ki + 1],
                                             scale=1.0 / 1024)
                        if qc == 0 and hp == 0 and ki < 6:
                            vproj(ki + 2)
                        elif qc == 1 and hp == 0 and ki >= 3:
                            epi(0, ki - 3, on_act=False)
                        elif qc == 1 and hp == 2 and ki < 3:
                            epi(0, ki + 5, on_act=False)
                        if jj == 0 and t >= 1:
                            av(t - 1, p2_tiles.pop(t - 1), stop=False)
                    av(3, p2_tiles.pop(3), stop=True)
                    # normalize: attn = at[0:64] / denom (denom row 64,
                    # plus BIG at masked q so those rows vanish).
                    # NB: reciprocal_approx_fast is broken on PSUM reads at
                    # partition offset 64 — bounce the row through SBUF.
                    for j, h in enumerate((hp, hp + 1)):
                        ro = 64 * (h % 2)
                        den = rot.tile([1, 512], f32, tag="den")
                        nc.vector.scalar_tensor_tensor(
                            den[:], ats[j][64:65, :], 0.0, bm_t[:, qs],
                            ALU.add, ALU.add)
                        rcp = rot.tile([1, 512], f32, tag="rcp")
                        nc.vector.reciprocal_approx_fast(rcp[:], den[:])
                        bc = rot.tile([64, 512], f32, tag="bc")
                        nc.gpsimd.partition_broadcast(bc[:], rcp[:])
                        nc.vector.tensor_mul(
                            attn[et][ro:ro + 64, qs], ats[j][0:64, :], bc[:])
                    # residual; stream O block out (host needs it)
                    nc.vector.tensor_add(ot[et][:, qs], qpt[et][:, qs],
                                         attn[et][:, qs])
                    nc.gpsimd.dma_start(oout[128 * et:128 * (et + 1), qs],
                                        ot[et][:, qs])

            # keep the PE warm through the last normalize chain so the
            # tail epilogue matmuls run at full clock
            junk = ps_big.tile([128, 1024], f32, tag="big", name="junk")
            for w in range(12):
                nc.tensor.matmul(junk[:, 0:512], warm_w[:], warm_x[:],
                                 start=(w == 0), stop=(w == 11))
            # qc1 partial output projection (tail; ACT is idle by now)
            for ec in range(8):
                epi(1, ec, on_act=(ec % 2 == 1))

    nc.compile()
    return nc


def _get_compiled():
    global _compiled
    if _compiled is None:
        _compiled = _build()
    return _compiled


def kernel(Q, K, mask_Q, mask_K, Wq, Wk, Wv, Wo):
    global LAST_RESULTS
    import ml_dtypes
    from concourse.bass_utils import run_bass_kernel_spmd

    bf = ml_dtypes.bfloat16
    f8t = ml_dtypes.float8_e4m3
    Q = np.asarray(Q, np.float32)
    K = np.asarray(K, np.float32)
    mask_Q = np.asarray(mask_Q, bool)
    mask_K = np.asarray(mask_K, bool)
    Wq = np.asarray(Wq, np.float32)
    Wk = np.asarray(Wk, np.float32)
    Wv = np.asarray(Wv, np.float32)
    Wo = np.asarray(Wo, np.float32)

    nc = _get_compiled()

    # host-side masking: zero masked token rows before projection
    Qz = np.where(mask_Q[:, :, None], 0.0, Q)
    Kz = np.where(mask_K[:, :, None], 0.0, K)

    wot = np.ascontiguousarray(Wo.T.astype(bf))
    in_maps = []
    for c in range(NCORES):
        b, g = c % 2, c // 2
        eb = slice(EB * g, EB * (g + 1))
        bias = np.where(mask_K[b], NEG, 0.0).astype(np.float32)
        def relay(a):
            # [1024, F] -> [128, 8*F] with dc-major columns
            F = a.shape[1]
            return np.ascontiguousarray(
                a.reshape(8, 128, F).transpose(1, 0, 2).reshape(128, 8 * F))

        in_maps.append({
            "qt": relay(Qz[b].T.astype(bf)),
            "kt": relay(Kz[b].T.astype(f8t)),
            "wq": relay(Wq[eb, :].T.astype(bf)),
            "wk": relay((Wk[eb, :].T * 32).astype(f8t)),
            "wv": relay((Wv[eb, :].T * 32).astype(f8t)),
            "wos": np.ascontiguousarray(wot[eb, :]),
            "bk": np.ascontiguousarray(bias.reshape(8, 128).T),
            "mvn": mask_Q[b].astype(bf)[None, :],
        })

    res = run_bass_kernel_spmd(nc, in_maps, core_ids=list(range(NCORES)),
                               trace=TRACE, **TRACE_KWARGS)
    LAST_RESULTS = res

    # unshard: O blocks -> O_full; sum partials per batch; relu + residual
    full = np.empty((B, L, D), np.float32)
    for b in range(B):
        Ob = np.empty((L, D), np.float32)
        ffb = np.zeros((L, D), np.float32)
        for g in range(4):
            c = 2 * g + b
            r = res.results[c]
            Ob[:, EB * g:EB * (g + 1)] = r["oout"].T.astype(np.float32)
            ffb += r["pout"].T.astype(np.float32)
        full[b] = Ob + np.maximum(ffb, 0.0)
    return full
